# revision 27
# baseline (speedup 1.0000x reference)
"""Causal self-attention (B=2,T=2048,C=1024,H=16) on 8 trn2 NeuronCores.

Sharding: core c handles batch b=c//4 and 4 heads (c%4)*4..+4 (tensor-parallel
over heads x data-parallel over batch). Each core computes:
  qkT = (Wqk_slice)^T @ x_b^T   (scores-scale folded into Wq)
  V   = x_b @ Wv_slice          (+ ones column per head for the softmax sum)
  S^T = K_h' Q_h -> exp -> causal select -> yT' = V''^T @ expS (65th row = sum)
  yT  = yT'[0:64] / yT'[64]     (1/l broadcast via a ones-matmul on the PE)
  outT_partial accumulated over the core's 256 channels (b_proj on cores g==0)
Host sums the 4 per-batch partials and transposes back. All matmuls in f32r
(TF32-class precision, 4x faster than fp32 on the PE).
"""
import sys

sys.path.insert(0, "/opt/trn_rl_repo")

import numpy as np

import concourse.bass as bass
import concourse.mybir as mybir
import concourse.tile as tile
from concourse import bacc
from concourse.bass_utils import run_bass_kernel_spmd

B, T, C, H, HD = 2, 2048, 1024, 16, 64
NCORES = 8
HPC = 4            # heads per core
CT = C // 128      # 8 C-tiles
TJ = T // 512      # 4 tok chunks
TT = T // 128      # 16 tok tiles
VW = HPC * (HD + 1)  # 260: V cols per core incl. ones column per head
F32 = mybir.dt.float32
F32R = mybir.dt.float32r

_CACHE = {}


def _emit(tc, nc, d):
    d_xT, d_wqk, d_wv, d_wp, d_bqk, d_bv, d_bp, d_ones, d_out = d
    with tc.tile_pool(name="const", bufs=1) as pc, \
         tc.tile_pool(name="qk", bufs=1) as pqk, \
         tc.tile_pool(name="vv", bufs=1) as pvv, \
         tc.tile_pool(name="yt", bufs=1) as pyt:
        bqk = pc.tile([128, 4], F32, tag="bqk")
        bv = pc.tile([128, VW], F32, tag="bv")
        ones = pc.tile([128, 64], F32R, tag="ones")
        nc.sync.dma_start(bqk[:], d_bqk)
        nc.sync.dma_start(bv[:], d_bv)
        nc.sync.dma_start(ones[:], d_ones)

        qkT = [pqk.tile([128, T], F32R, tag=f"qk{i}", name=f"qkT{i}") for i in range(4)]
        V = [pvv.tile([128, VW], F32R, tag=f"v{i}", name=f"V{i}") for i in range(TT)]
        yT = [pyt.tile([128, T], F32R, tag=f"y{i}", name=f"yT{i}") for i in range(2)]

        # ---------------- stage A: QKV projections ----------------
        with tc.tile_pool(name="w_in", bufs=1) as pw, \
             tc.tile_pool(name="x_in", bufs=1) as px, \
             tc.tile_pool(name="psA", bufs=4, space="PSUM") as psA:
            wqk = pw.tile([128, CT * 512], F32R, tag="wqk")
            wv = pw.tile([128, CT * VW], F32R, tag="wv")
            xT = px.tile([128, CT * T], F32R, tag="xT")
            # first accumulation group (tj=0) needs wqk/wv/xT for every ct:
            # interleave so it can start ~2.5us in
            for ct in range(CT):
                nc.sync.dma_start(wqk[:, ct * 512:(ct + 1) * 512],
                                  d_wqk[:, ct * 512:(ct + 1) * 512])
                nc.sync.dma_start(
                    xT[:, ct * T:ct * T + 512], d_xT[:, ct * T:ct * T + 512])
            for ct in range(CT):
                nc.sync.dma_start(wv[:, ct * VW:(ct + 1) * VW],
                                  d_wv[:, ct * VW:(ct + 1) * VW])
            for tj in range(1, TJ):
                for ct in range(CT):
                    nc.sync.dma_start(
                        xT[:, ct * T + tj * 512:ct * T + (tj + 1) * 512],
                        d_xT[:, ct * T + tj * 512:ct * T + (tj + 1) * 512])

            # proj weights/bias loaded after the latency-critical stage-A data
            bp = pc.tile([128, 8], F32, tag="bp")
            nc.sync.dma_start(bp[:], d_bp)
            wp = pc.tile([128, 2 * C], F32R, tag="wp")
            nc.sync.dma_start(wp[:], d_wp)

            for tj in range(TJ):
                # q,k for 4 heads: M-tiles [q01, k01, q23, k23]
                for mo in range(4):
                    ps = psA.tile([128, 512], F32, tag="psqk")
                    for ct in range(CT):
                        nc.tensor.matmul(
                            ps[:],
                            wqk[:, ct * 512 + mo * 128:ct * 512 + (mo + 1) * 128],
                            xT[:, ct * T + tj * 512:ct * T + (tj + 1) * 512],
                            start=(ct == 0), stop=(ct == CT - 1))
                    nc.vector.tensor_scalar_add(
                        qkT[mo][:, tj * 512:(tj + 1) * 512], ps[:], bqk[:, mo:mo + 1])
                # V natural layout for tok tiles of this chunk
                for tt in range(4 * tj, 4 * tj + 4):
                    psv = psA.tile([128, VW], F32, tag="psv")
                    for ct in range(CT):
                        nc.tensor.matmul(
                            psv[:],
                            xT[:, ct * T + tt * 128:ct * T + (tt + 1) * 128],
                            wv[:, ct * VW:(ct + 1) * VW],
                            start=(ct == 0), stop=(ct == CT - 1))
                    nc.vector.tensor_add(V[tt][:], psv[:], bv[:])

        # ---------------- stage B + C: attention, proj interleaved per qj ----
        # scores for k-tile pairs (kt, kt+1) land in one 2-bank PSUM tile so
        # a single ACT exp covers both; causal mask via gpsimd affine_select
        # (keep where p + 128*r <= f, else fill 0) on the diagonal tiles only.
        # After both head-pairs finish a tok chunk qj, its output projection
        # runs immediately so stage C overlaps later attention chunks.
        with tc.tile_pool(name="psS", bufs=2, space="PSUM") as psS, \
             tc.tile_pool(name="psY", bufs=3, space="PSUM") as psY, \
             tc.tile_pool(name="psC", bufs=1, space="PSUM") as psC, \
             tc.tile_pool(name="ex", bufs=6) as pex, \
             tc.tile_pool(name="nrm", bufs=4) as pn, \
             tc.tile_pool(name="po", bufs=4) as po:
            for qj in range(TJ):
                nkt = 4 * qj + 4
                for hp in range(2):
                    qt, kt_ = qkT[2 * hp], qkT[2 * hp + 1]
                    yps = [psY.tile([65, 512], F32, tag="yps", name=f"yps{_i}")
                           for _i in range(2)]
                    for kt0 in range(0, nkt, 2):
                        es = []
                        for lh in range(2):
                            # the two heads' score matmuls sit in different PE
                            # row groups (rows 0-63 / 64-127) and run
                            # concurrently on hardware
                            lo, hi = 64 * lh, 64 * lh + 64
                            s = psS.tile([128, 1024], F32, tag="s")
                            for ki in range(2):
                                kti = kt0 + ki
                                nc.tensor.matmul(
                                    s[:, ki * 512:(ki + 1) * 512],
                                    kt_[lo:hi, kti * 128:(kti + 1) * 128],
                                    qt[lo:hi, qj * 512:(qj + 1) * 512],
                                    start=True, stop=True)
                            e = pex.tile([128, 1024], F32R, tag="e")
                            nc.scalar.activation(
                                e[:], s[:], mybir.ActivationFunctionType.Exp)
                            for ki in range(2):
                                diag_r = kt0 + ki - 4 * qj
                                if diag_r >= 0:
                                    nc.gpsimd.affine_select(
                                        out=e[:, ki * 512:(ki + 1) * 512],
                                        in_=e[:, ki * 512:(ki + 1) * 512],
                                        compare_op=mybir.AluOpType.is_ge,
                                        fill=0.0,
                                        base=-128 * diag_r,
                                        pattern=[[1, 512]],
                                        channel_multiplier=-1)
                            es.append(e)
                        for lh in range(2):
                            h_loc = 2 * hp + lh
                            for ki in range(2):
                                kti = kt0 + ki
                                nc.tensor.matmul(
                                    yps[lh][:],
                                    V[kti][:, h_loc * 65:h_loc * 65 + 65],
                                    es[lh][:, ki * 512:(ki + 1) * 512],
                                    start=(kti == 0), stop=(kti == nkt - 1))
                    for lh in range(2):
                        # normalize: 1/l broadcast down 64 partitions via PE
                        rc = pn.tile([65, 512], F32R, tag="rc")
                        with nc.allow_low_precision(reason="1/l rounded to f32r for PE broadcast"):
                            nc.vector.reciprocal(rc[64:65, :], yps[lh][64:65, :])
                        bcp = psC.tile([64, 512], F32, tag="pps", name="bcp")
                        nc.tensor.matmul(bcp[:], ones[64:65, :], rc[64:65, :],
                                         start=True, stop=True)
                        bc = pn.tile([64, 512], F32, tag="bc")
                        nc.vector.tensor_copy(bc[:], bcp[:])
                        if lh == 0:
                            # partitions already aligned: write yT directly
                            nc.vector.tensor_mul(
                                yT[hp][0:64, qj * 512:(qj + 1) * 512],
                                yps[lh][0:64, :], bc[:])
                        else:
                            ty = pn.tile([64, 512], F32R, tag="ty")
                            nc.vector.tensor_mul(ty[:], yps[lh][0:64, :], bc[:])
                            nc.sync.dma_start(
                                yT[hp][64:128, qj * 512:(qj + 1) * 512], ty[:])
                # output projection for this tok chunk
                for mo in range(8):
                    pps = psC.tile([128, 512], F32, tag="pps")
                    for kt2 in range(2):
                        nc.tensor.matmul(
                            pps[:],
                            wp[:, kt2 * C + mo * 128:kt2 * C + (mo + 1) * 128],
                            yT[kt2][:, qj * 512:(qj + 1) * 512],
                            start=(kt2 == 0), stop=(kt2 == 1))
                    ot = po.tile([128, 512], F32, tag="ot")
                    nc.vector.tensor_scalar_add(ot[:], pps[:], bp[:, mo:mo + 1])
                    nc.sync.dma_start(
                        d_out[:, mo * T + qj * 512:mo * T + (qj + 1) * 512], ot[:])


def _build(reps=1):
    nc = bacc.Bacc("TRN2", target_bir_lowering=False, debug=False)
    d = (
        nc.dram_tensor("xT", [128, CT * T], F32R, kind="ExternalInput").ap(),
        nc.dram_tensor("wqk", [128, CT * 512], F32R, kind="ExternalInput").ap(),
        nc.dram_tensor("wv", [128, CT * VW], F32R, kind="ExternalInput").ap(),
        nc.dram_tensor("wp", [128, 2 * C], F32R, kind="ExternalInput").ap(),
        nc.dram_tensor("bqk", [128, 4], F32, kind="ExternalInput").ap(),
        nc.dram_tensor("bv", [128, VW], F32, kind="ExternalInput").ap(),
        nc.dram_tensor("bp", [128, 8], F32, kind="ExternalInput").ap(),
        nc.dram_tensor("ones", [128, 64], F32R, kind="ExternalInput").ap(),
        nc.dram_tensor("outT", [128, 8 * T], F32, kind="ExternalOutput").ap(),
    )
    with tile.TileContext(nc) as tc:
        for rep in range(reps):
            if rep:
                tc.strict_bb_all_engine_barrier()
            _emit(tc, nc, d)
    nc.compile()
    return nc


def _sb(a):
    """[128k, n] -> SBUF layout [128, k*n] (k-tile-major along free dim)."""
    k = a.shape[0] // 128
    return np.ascontiguousarray(
        a.reshape(k, 128, a.shape[1]).transpose(1, 0, 2).reshape(128, -1)
    ).astype(np.float32)


def _prep_in_maps(inputs):
    x = np.asarray(inputs["x"], np.float32)
    W_attn = np.asarray(inputs["W_attn"], np.float32)
    b_attn = np.asarray(inputs["b_attn"], np.float32)
    W_proj = np.asarray(inputs["W_proj"], np.float32)
    b_proj = np.asarray(inputs["b_proj"], np.float32)

    scale = 1.0 / np.sqrt(HD)
    ones = np.ones((128, 64), np.float32)

    in_maps = []
    for c in range(NCORES):
        b, g = divmod(c, 4)
        heads = [4 * g + i for i in range(HPC)]
        xT = _sb(np.ascontiguousarray(x[b].T))                      # [128, 8*2048]

        wq = [W_attn[:, h * HD:(h + 1) * HD] * scale for h in heads]
        wk = [W_attn[:, C + h * HD:C + (h + 1) * HD] for h in heads]
        wqk = np.concatenate(
            [wq[0], wq[1], wk[0], wk[1], wq[2], wq[3], wk[2], wk[3]], axis=1)
        wqk = _sb(wqk)                                              # [128, 8*512]

        wv = np.zeros((C, VW), np.float32)
        for i, h in enumerate(heads):
            wv[:, i * 65:i * 65 + 64] = W_attn[:, 2 * C + h * HD:2 * C + (h + 1) * HD]
        wv = _sb(wv)                                                # [128, 8*260]

        wp = np.zeros((128, 2 * C), np.float32)
        for kt2 in range(2):
            rows = np.concatenate(
                [W_proj[heads[2 * kt2 + j] * HD:(heads[2 * kt2 + j] + 1) * HD, :]
                 for j in range(2)], axis=0)                        # [128, 1024]
            wp[:, kt2 * C:(kt2 + 1) * C] = rows

        bqk = np.zeros((128, 4), np.float32)
        for i2 in range(2):   # head pair
            for j in range(2):
                h = heads[2 * i2 + j]
                bqk[64 * j:64 * j + 64, 2 * i2] = b_attn[h * HD:(h + 1) * HD] * scale
                bqk[64 * j:64 * j + 64, 2 * i2 + 1] = b_attn[C + h * HD:C + (h + 1) * HD]

        bv = np.zeros(VW, np.float32)
        for i, h in enumerate(heads):
            bv[i * 65:i * 65 + 64] = b_attn[2 * C + h * HD:2 * C + (h + 1) * HD]
            bv[i * 65 + 64] = 1.0
        bv = np.tile(bv[None, :], (128, 1)).astype(np.float32)

        bp = np.zeros((128, 8), np.float32)
        if g == 0:
            bp[:] = b_proj.reshape(8, 128).T

        in_maps.append({"xT": xT, "wqk": wqk, "wv": wv, "wp": wp,
                        "bqk": bqk, "bv": bv, "bp": bp, "ones": ones})
    return in_maps


def kernel(x, W_attn, b_attn, W_proj, b_proj):
    in_maps = _prep_in_maps(dict(x=x, W_attn=W_attn, b_attn=b_attn,
                                 W_proj=W_proj, b_proj=b_proj))
    if "nc" not in _CACHE:
        _CACHE["nc"] = _build()
    nc = _CACHE["nc"]
    res = run_bass_kernel_spmd(nc, in_maps, core_ids=list(range(NCORES)))

    out = np.zeros((B, T, C), np.float32)
    for c in range(NCORES):
        b = c // 4
        oT = res.results[c]["outT"]                                 # [128, 8*2048]
        oT = oT.reshape(128, 8, T).transpose(1, 0, 2).reshape(C, T)  # [C, T]
        out[b] += oT.T
    return out
